# revision 18
# baseline (speedup 1.0000x reference)
"""Trainium2 kernel for nn_Circuit_41936060678727.

The reference is a 10-qubit real-amplitude circuit (CNOT ladders + RY
rotations) applied to an amplitude-embedded batch, measured with PauliZ on
each of the 10 wires.  Every gate is linear in the state, so the whole
8-layer circuit collapses to one fixed 1024x1024 orthogonal matrix M that
depends only on `params` (8x10).  With x padded to 1024 and L2-normalized:

    out[b, p] = sum_z (x[b] @ M[:784, :])[z]^2 * sign_p(z) / ||x[b]||^2

because M is orthogonal the norm comes for free as an extra all-ones column
of the sign matrix, and the ratio is invariant to any per-tensor scaling of
the matmul inputs -- which buys the precision headroom for fp8.

Device pipeline per core (batch 16384 data-parallel over 8 cores, 2048 each):
    y = W8^T @ d8 (+ tail)  K rows 0..767: fp8e4m3 DoubleRow matmuls
                            (K=256 per pass), d8 = e4m3(32*(x-0.5)),
                            W8 = e4m3(16*W).  K rows 768..783 plus the
                            exact mean row (256*colsum(W), from centering)
                            ride in one fp16 K=17 matmul per (z, group).
    sq = y^2                pure squares, alternating vector/scalar engine
    o  = Zaug^T @ sq        bf16 matmuls accumulated into a single PSUM
                            bank at 4 col-group quadrants (one per batch
                            group of 512); the 4 quadrant matmuls pack
                            concurrently in the PE array.
Host: out = (o[:10] / o[10]) per group, concat cores.

Centering x around 0.5 halves the fp8 quantization error of the dominant
DC component; the fp16 mean row removes it from the fp8 path entirely.
"""

import numpy as np
import ml_dtypes

N_QUBITS = 10
DIM = 1 << N_QUBITS          # 1024
N_OUT = 10
D_IN = 784
B_TOTAL = 16384
N_CORES = 8
B_CORE = B_TOTAL // N_CORES  # 2048
GROUP = 512                  # batch columns per matmul (one PSUM bank, fp32)
N_GROUPS = B_CORE // GROUP   # 4
NJ = 3                       # full DoubleRow K-chunks of 256 (rows 0..767)
K_TAIL = 17                  # fp16 tail: rows 768..783 + mean row
Z_CHUNK = 128
N_ZCH = DIM // Z_CHUNK       # 8
ZCOLS = 16                   # cols 0..9 = PauliZ signs, 10 = ones, 11..15 pad
SX = 32.0                    # fp8 scale for centered x
SW = 16.0                    # fp8 scale for W

F8 = ml_dtypes.float8_e4m3   # TRN2 float8e4 (max normal 240)
BF16 = ml_dtypes.bfloat16


# ----------------------------------------------------------------------------
# Host-side precompute: collapse the circuit to W = M[:784, :]
# ----------------------------------------------------------------------------

def _apply_ry(S, theta, q):
    B = S.shape[0]
    left, right = 1 << q, 1 << (N_QUBITS - q - 1)
    s = S.reshape(B, left, 2, right)
    c, sn = np.cos(theta / 2), np.sin(theta / 2)
    s0 = c * s[:, :, 0] - sn * s[:, :, 1]
    s1 = sn * s[:, :, 0] + c * s[:, :, 1]
    return np.stack([s0, s1], axis=2).reshape(B, DIM)


def _apply_cnot(S, q):
    B = S.shape[0]
    left, right = 1 << q, 1 << (N_QUBITS - q - 2)
    s = S.reshape(B, left, 2, 2, right)
    s = np.concatenate([s[:, :, :1], np.flip(s[:, :, 1:], axis=3)], axis=2)
    return s.reshape(B, DIM)


def _build_W(params):
    """Circuit applied to basis rows e_0..e_783 -> W[784, 1024], fp64."""
    w = np.pi * np.tanh(params.astype(np.float64))
    S = np.zeros((D_IN, DIM), dtype=np.float64)
    S[np.arange(D_IN), np.arange(D_IN)] = 1.0
    for l in range(params.shape[0]):
        for start in (0, 1):
            for i in range(start, N_QUBITS - 1, 2):
                S = _apply_cnot(S, i)
        for i in range(N_QUBITS):
            S = _apply_ry(S, w[l, i], i)
    return S


def _build_Z():
    z = np.arange(DIM)
    Z = np.zeros((DIM, ZCOLS), dtype=np.float32)
    for p in range(N_OUT):
        Z[:, p] = 1.0 - 2.0 * ((z >> (N_QUBITS - 1 - p)) & 1)
    Z[:, N_OUT] = 1.0
    # device layout [128, 8*16]: z-chunk zc rows zc*128.. at cols zc*16..
    Zd = Z.reshape(N_ZCH, Z_CHUNK, ZCOLS).transpose(1, 0, 2).reshape(Z_CHUNK, -1)
    return np.ascontiguousarray(Zd).astype(BF16)


# ----------------------------------------------------------------------------
# Bass program (identical SPMD program on all 8 cores)
# ----------------------------------------------------------------------------

_NC_CACHE = {}
TRACE = False           # test harness can flip this for profiling
LAST_RESULTS = None


def _build_bass():
    from contextlib import ExitStack

    import concourse.tile as tile
    from concourse import bacc, mybir

    f32 = mybir.dt.float32
    f8 = mybir.dt.float8e4
    f16 = mybir.dt.float16
    bf16 = mybir.dt.bfloat16
    DR = mybir.MatmulPerfMode.DoubleRow

    nc = bacc.Bacc(
        "TRN2", target_bir_lowering=False, debug=False, num_devices=N_CORES
    )
    xt_d = nc.declare_dram_parameter("xt", [128, NJ, 2, B_CORE], f8, isOutput=False)
    wt_d = nc.declare_dram_parameter(
        "wt", [128, N_ZCH, NJ, 2, Z_CHUNK], f8, isOutput=False
    )
    # fp16 tail operands packed in one tensor: cols 0..1023 = weights
    # (16 W rows + mean row), cols 1024.. = rhs (16 x rows + ones row)
    tl_d = nc.declare_dram_parameter(
        "tl", [K_TAIL, DIM + B_CORE], f16, isOutput=False
    )
    zt_d = nc.declare_dram_parameter(
        "zt", [Z_CHUNK, N_ZCH * ZCOLS], bf16, isOutput=False
    )
    out_d = nc.declare_dram_parameter("out", [128, GROUP], f32, isOutput=True)

    with ExitStack() as ctx:
        tc = ctx.enter_context(tile.TileContext(nc))
        wpool = ctx.enter_context(tc.tile_pool(name="w", bufs=1))
        xpool = ctx.enter_context(tc.tile_pool(name="x", bufs=1))
        zpool = ctx.enter_context(tc.tile_pool(name="z", bufs=1))
        sqpool = ctx.enter_context(tc.tile_pool(name="sq", bufs=12))
        tmppool = ctx.enter_context(tc.tile_pool(name="tmp", bufs=3))
        opool = ctx.enter_context(tc.tile_pool(name="osb", bufs=1))
        # 7 rotating y banks + 1 persistent output bank = all 8 PSUM banks
        pypool = ctx.enter_context(tc.tile_pool(name="py", bufs=7, space="PSUM"))
        popool = ctx.enter_context(tc.tile_pool(name="po", bufs=1, space="PSUM"))

        # PE pre-warm: dependency-free full-K bf16 matmuls keep the tensor
        # engine ARRAY visibly busy during the DMA prefix so the HAM
        # clock-gate releases (1.2 -> 2.4 GHz) before real work arrives.
        # (K=1 matmuls barely register as activity to the HAM.)
        warm_in = opool.tile([128, 384], bf16, name="warm_in")
        nc.any.memset(warm_in[:], 0.0)
        warm_ps = pypool.tile([128, 256], f32, name="warm_ps", tag="py")
        for i in range(14):
            nc.tensor.matmul(
                warm_ps[:],
                lhsT=warm_in[:, 0:128],
                rhs=warm_in[:, 128:384],
                start=True,
                stop=True,
                skip_group_check=True,
            )

        # Input DMAs on one HWDGE ring.  Descriptor generation costs
        # ~700ns per dma_start serialized on the sync queue, so: few DMAs,
        # most-critical first (z=0 j=0 operands), in consumption order.
        x0_sb = xpool.tile([128, 2, B_CORE], f8, name="x0_sb")
        nc.sync.dma_start(x0_sb[:], xt_d[:, 0])
        w0_sb = wpool.tile([128, NJ, 2, Z_CHUNK], f8, name="w0_sb")
        nc.sync.dma_start(w0_sb[:], wt_d[:, 0])
        x1_sb = xpool.tile([128, 2, B_CORE], f8, name="x1_sb")
        nc.sync.dma_start(x1_sb[:], xt_d[:, 1])
        x2_sb = xpool.tile([128, 2, B_CORE], f8, name="x2_sb")
        nc.sync.dma_start(x2_sb[:], xt_d[:, 2])
        tl_sb = wpool.tile([K_TAIL, DIM + B_CORE], f16, name="tl_sb")
        nc.sync.dma_start(tl_sb[:], tl_d[:])
        wr13_sb = wpool.tile([128, 3, NJ, 2, Z_CHUNK], f8, name="wr13_sb")
        nc.sync.dma_start(wr13_sb[:], wt_d[:, 1:4])
        wr47_sb = wpool.tile([128, 4, NJ, 2, Z_CHUNK], f8, name="wr47_sb")
        nc.sync.dma_start(wr47_sb[:], wt_d[:, 4:8])
        z_sb = zpool.tile([Z_CHUNK, N_ZCH * ZCOLS], bf16, name="z_sb")
        nc.sync.dma_start(z_sb[:], zt_d[:])

        po = popool.tile([128, GROUP], f32, name="po")
        # zero the never-written partitions so the single full-bank drain
        # copy below reads defined data everywhere
        nc.vector.memset(po[:], 0.0)
        out_sb = opool.tile([128, GROUP], f32, name="out_sb")

        def gsl(g):
            return slice(g * GROUP, (g + 1) * GROUP)

        def wslice(z, j):
            if z == 0:
                return w0_sb[:, j]
            if z <= 3:
                return wr13_sb[:, z - 1, j]
            return wr47_sb[:, z - 4, j]

        def xslice(j, g):
            return [x0_sb, x1_sb, x2_sb][j][:, :, gsl(g)]

        # z-outer / j / g-inner: each DoubleRow weight load serves the 4
        # batch-group matmuls, hiding LDWEIGHTS under streaming.  mm2 for
        # z is deferred until after z+2's matmul block: by then its squares
        # are long done, so the semaphore wait attached to its LDWEIGHTS
        # (move_matmul_waits_to_ldweights) never head-of-line-blocks the
        # next j-group's weight load.
        pending = []
        for z in range(N_ZCH):
            pys = [
                pypool.tile([Z_CHUNK, GROUP], f32, tag="py", name=f"py_{z}_{g}")
                for g in range(N_GROUPS)
            ]
            for j in range(NJ):
                for g in range(N_GROUPS):
                    nc.tensor.matmul(
                        pys[g][:],
                        lhsT=wslice(z, j),
                        rhs=xslice(j, g),
                        start=(j == 0),
                        stop=False,
                        perf_mode=DR,
                        skip_group_check=True,
                    )
            for g in range(N_GROUPS):
                nc.tensor.matmul(
                    pys[g][:],
                    lhsT=tl_sb[:, z * Z_CHUNK:(z + 1) * Z_CHUNK],
                    rhs=tl_sb[:, DIM + g * GROUP:DIM + (g + 1) * GROUP],
                    start=False,
                    stop=True,
                    skip_group_check=True,
                )
            if len(pending) == 2:
                pending.pop(0)()
            # Squares split across engines so PSUM banks free in parallel:
            # DVE can't read both operands from PSUM, so it copies to SBUF
            # first (the copy is what releases the py bank) and multiplies
            # there; the scalar engine squares straight from PSUM.
            sqs = [None] * N_GROUPS
            tmps = {}
            for g in (0, 2):
                tmp = tmppool.tile([Z_CHUNK, GROUP], f32, tag="tmp")
                nc.vector.tensor_copy(tmp[:], pys[g][:])
                tmps[g] = tmp
            for g in (1, 3):
                sq = sqpool.tile([Z_CHUNK, GROUP], bf16, tag="sq")
                nc.scalar.square(sq[:], pys[g][:])
                sqs[g] = sq
            for g in (0, 2):
                sq = sqpool.tile([Z_CHUNK, GROUP], bf16, tag="sq")
                nc.vector.tensor_mul(sq[:], tmps[g][:], tmps[g][:])
                sqs[g] = sq

            def make_mm2(z, sqs):
                def mm2():
                    for g in range(N_GROUPS):
                        nc.tensor.matmul(
                            po[32 * g:32 * g + N_OUT + 1, :],
                            lhsT=z_sb[:, z * ZCOLS:z * ZCOLS + N_OUT + 1],
                            rhs=sqs[g][:],
                            start=(z == 0),
                            stop=(z == N_ZCH - 1),
                            skip_group_check=True,
                            tile_position=(0, 32 * g),
                        )
                return mm2

            pending.append(make_mm2(z, sqs))
        for p in pending:
            p()

        # Final drain: one full-bank PSUM->SBUF copy, one output DMA.
        nc.vector.tensor_copy(out_sb[:], po[:])
        nc.sync.dma_start(out_d[:], out_sb[:])

    nc.finalize()
    return nc


def _get_nc():
    if "nc" not in _NC_CACHE:
        _NC_CACHE["nc"] = _build_bass()
    return _NC_CACHE["nc"]


# ----------------------------------------------------------------------------
# Host-side packing
# ----------------------------------------------------------------------------

def _pack_weights(params):
    W = _build_W(np.asarray(params, dtype=np.float32))        # [784, 1024] fp64
    W8 = (W * SW).astype(np.float32).astype(F8)               # [784, 1024]
    # wt[p, z, j, i, m] = W8[256j + 128i + p, 128z + m]
    wt = np.ascontiguousarray(
        np.asarray(W8[:NJ * 256]).reshape(NJ, 2, 128, N_ZCH, Z_CHUNK)
        .transpose(2, 3, 0, 1, 4)
    )
    # fp16 tail weights: rows 768..783 scaled by SW, plus the exact mean
    # row SX*SW*0.5*colsum(W) from centering x around 0.5
    tlw = np.zeros((K_TAIL, DIM), dtype=np.float16)
    tlw[:16] = (W[NJ * 256:] * SW).astype(np.float16)
    tlw[16] = (SX * SW * 0.5 * W.sum(axis=0)).astype(np.float16)
    return wt, tlw


def _pack_x(xc, tlw):
    """xc [2048, 784] fp32 -> (xt [128, 3, 2, 2048] fp8, tl [17, 3072] fp16)."""
    d = (xc - 0.5) * SX
    d8 = d[:, :NJ * 256].astype(F8)
    xt = np.ascontiguousarray(
        np.asarray(d8).reshape(B_CORE, NJ, 2, 128).transpose(3, 1, 2, 0)
    )
    tl = np.empty((K_TAIL, DIM + B_CORE), dtype=np.float16)
    tl[:, :DIM] = tlw
    tl[:16, DIM:] = d[:, NJ * 256:].astype(np.float16).T
    tl[16, DIM:] = 1.0
    return xt, tl


# ----------------------------------------------------------------------------
# Entry point
# ----------------------------------------------------------------------------

def kernel(input, params):
    global LAST_RESULTS
    from concourse.bass_utils import run_bass_kernel_spmd

    x = np.asarray(input, dtype=np.float32)
    wt, tlw = _pack_weights(params)
    zt = _build_Z()

    nc = _get_nc()
    in_maps = []
    for c in range(N_CORES):
        xt, tl = _pack_x(x[c * B_CORE:(c + 1) * B_CORE], tlw)
        in_maps.append({"xt": xt, "wt": wt, "tl": tl, "zt": zt})

    res = run_bass_kernel_spmd(nc, in_maps, list(range(N_CORES)), trace=TRACE)
    LAST_RESULTS = res

    outs = []
    for c in range(N_CORES):
        o = res.results[c]["out"].reshape(N_GROUPS, 32, GROUP)
        outs.append(
            (o[:, :N_OUT, :] / o[:, N_OUT:N_OUT + 1, :])
            .transpose(0, 2, 1).reshape(B_CORE, N_OUT)
        )
    return np.ascontiguousarray(np.concatenate(outs, axis=0).astype(np.float32))


# revision 19
# speedup vs baseline: 1.1142x; 1.1142x over previous
"""Trainium2 kernel for nn_Circuit_41936060678727.

The reference is a 10-qubit real-amplitude circuit (CNOT ladders + RY
rotations) applied to an amplitude-embedded batch, measured with PauliZ on
each of the 10 wires.  Every gate is linear in the state, so the whole
8-layer circuit collapses to one fixed 1024x1024 orthogonal matrix M that
depends only on `params` (8x10).  With x padded to 1024 and L2-normalized:

    out[b, p] = sum_z (x[b] @ M[:784, :])[z]^2 * sign_p(z) / ||x[b]||^2

because M is orthogonal the norm comes for free as an extra all-ones column
of the sign matrix, and the ratio is invariant to any per-tensor scaling of
the matmul inputs -- which buys the precision headroom for fp8.

Device pipeline per core (batch 16384 data-parallel over 8 cores, 2048 each):
    y = W8^T @ d8 (+ tail)  K rows 0..767: fp8e4m3 DoubleRow matmuls
                            (K=256 per pass), d8 = e4m3(32*(x-0.5)),
                            W8 = e4m3(16*W).  K rows 768..783 plus the
                            exact mean row (256*colsum(W), from centering)
                            ride in one fp16 K=17 matmul per (z, group).
    sq = y^2                pure squares, alternating vector/scalar engine
    o  = Zaug^T @ sq        bf16 matmuls accumulated into a single PSUM
                            bank at 4 col-group quadrants (one per batch
                            group of 512); the 4 quadrant matmuls pack
                            concurrently in the PE array.
Host: out = (o[:10] / o[10]) per group, concat cores.

Centering x around 0.5 halves the fp8 quantization error of the dominant
DC component; the fp16 mean row removes it from the fp8 path entirely.
"""

import numpy as np
import ml_dtypes

N_QUBITS = 10
DIM = 1 << N_QUBITS          # 1024
N_OUT = 10
D_IN = 784
B_TOTAL = 16384
N_CORES = 8
B_CORE = B_TOTAL // N_CORES  # 2048
GROUP = 512                  # batch columns per matmul (one PSUM bank, fp32)
N_GROUPS = B_CORE // GROUP   # 4
NJ = 3                       # full DoubleRow K-chunks of 256 (rows 0..767)
K_TAIL = 17                  # fp16 tail: rows 768..783 + mean row
Z_CHUNK = 128
N_ZCH = DIM // Z_CHUNK       # 8
ZCOLS = 16                   # cols 0..9 = PauliZ signs, 10 = ones, 11..15 pad
SX = 32.0                    # fp8 scale for centered x
SW = 16.0                    # fp8 scale for W

F8 = ml_dtypes.float8_e4m3   # TRN2 float8e4 (max normal 240)
BF16 = ml_dtypes.bfloat16


# ----------------------------------------------------------------------------
# Host-side precompute: collapse the circuit to W = M[:784, :]
# ----------------------------------------------------------------------------

def _apply_ry(S, theta, q):
    B = S.shape[0]
    left, right = 1 << q, 1 << (N_QUBITS - q - 1)
    s = S.reshape(B, left, 2, right)
    c, sn = np.cos(theta / 2), np.sin(theta / 2)
    s0 = c * s[:, :, 0] - sn * s[:, :, 1]
    s1 = sn * s[:, :, 0] + c * s[:, :, 1]
    return np.stack([s0, s1], axis=2).reshape(B, DIM)


def _apply_cnot(S, q):
    B = S.shape[0]
    left, right = 1 << q, 1 << (N_QUBITS - q - 2)
    s = S.reshape(B, left, 2, 2, right)
    s = np.concatenate([s[:, :, :1], np.flip(s[:, :, 1:], axis=3)], axis=2)
    return s.reshape(B, DIM)


def _build_W(params):
    """Circuit applied to basis rows e_0..e_783 -> W[784, 1024], fp64."""
    w = np.pi * np.tanh(params.astype(np.float64))
    S = np.zeros((D_IN, DIM), dtype=np.float64)
    S[np.arange(D_IN), np.arange(D_IN)] = 1.0
    for l in range(params.shape[0]):
        for start in (0, 1):
            for i in range(start, N_QUBITS - 1, 2):
                S = _apply_cnot(S, i)
        for i in range(N_QUBITS):
            S = _apply_ry(S, w[l, i], i)
    return S


def _build_Z():
    z = np.arange(DIM)
    Z = np.zeros((DIM, ZCOLS), dtype=np.float32)
    for p in range(N_OUT):
        Z[:, p] = 1.0 - 2.0 * ((z >> (N_QUBITS - 1 - p)) & 1)
    Z[:, N_OUT] = 1.0
    # device layout [128, 8*16]: z-chunk zc rows zc*128.. at cols zc*16..
    Zd = Z.reshape(N_ZCH, Z_CHUNK, ZCOLS).transpose(1, 0, 2).reshape(Z_CHUNK, -1)
    return np.ascontiguousarray(Zd).astype(BF16)


# ----------------------------------------------------------------------------
# Bass program (identical SPMD program on all 8 cores)
# ----------------------------------------------------------------------------

_NC_CACHE = {}
TRACE = False           # test harness can flip this for profiling
LAST_RESULTS = None


def _build_bass():
    from contextlib import ExitStack

    import concourse.tile as tile
    from concourse import bacc, mybir

    f32 = mybir.dt.float32
    f8 = mybir.dt.float8e4
    f16 = mybir.dt.float16
    bf16 = mybir.dt.bfloat16
    DR = mybir.MatmulPerfMode.DoubleRow

    nc = bacc.Bacc(
        "TRN2", target_bir_lowering=False, debug=False, num_devices=N_CORES
    )
    xt_d = nc.declare_dram_parameter("xt", [128, NJ, 2, B_CORE], f8, isOutput=False)
    wt_d = nc.declare_dram_parameter(
        "wt", [128, N_ZCH, NJ, 2, Z_CHUNK], f8, isOutput=False
    )
    # fp16 tail operands packed in one tensor: cols 0..1023 = weights
    # (16 W rows + mean row), cols 1024.. = rhs (16 x rows + ones row)
    tl_d = nc.declare_dram_parameter(
        "tl", [K_TAIL, DIM + B_CORE], f16, isOutput=False
    )
    zt_d = nc.declare_dram_parameter(
        "zt", [Z_CHUNK, N_ZCH * ZCOLS], bf16, isOutput=False
    )
    out_d = nc.declare_dram_parameter("out", [128, GROUP], f32, isOutput=True)

    with ExitStack() as ctx:
        tc = ctx.enter_context(tile.TileContext(nc))
        wpool = ctx.enter_context(tc.tile_pool(name="w", bufs=1))
        xpool = ctx.enter_context(tc.tile_pool(name="x", bufs=1))
        zpool = ctx.enter_context(tc.tile_pool(name="z", bufs=1))
        sqpool = ctx.enter_context(tc.tile_pool(name="sq", bufs=12))
        tmppool = ctx.enter_context(tc.tile_pool(name="tmp", bufs=3))
        opool = ctx.enter_context(tc.tile_pool(name="osb", bufs=1))
        # 7 rotating y banks + 1 persistent output bank = all 8 PSUM banks
        pypool = ctx.enter_context(tc.tile_pool(name="py", bufs=7, space="PSUM"))
        popool = ctx.enter_context(tc.tile_pool(name="po", bufs=1, space="PSUM"))

        # PE pre-warm: dependency-free full-K bf16 matmuls keep the tensor
        # engine ARRAY visibly busy during the DMA prefix so the HAM
        # clock-gate releases (1.2 -> 2.4 GHz) before real work arrives.
        # (K=1 matmuls barely register as activity to the HAM.)
        warm_in = opool.tile([128, 384], bf16, name="warm_in")
        nc.any.memset(warm_in[:], 0.0)
        warm_ps = pypool.tile([128, 256], f32, name="warm_ps", tag="py")
        for i in range(14):
            nc.tensor.matmul(
                warm_ps[:],
                lhsT=warm_in[:, 0:128],
                rhs=warm_in[:, 128:384],
                start=True,
                stop=True,
                skip_group_check=True,
            )

        # Input DMAs on one HWDGE ring.  Descriptor generation costs
        # ~700ns per dma_start serialized on the sync queue, so: few DMAs,
        # most-critical first (z=0 j=0 operands), in consumption order.
        x0_sb = xpool.tile([128, 2, B_CORE], f8, name="x0_sb")
        nc.sync.dma_start(x0_sb[:], xt_d[:, 0])
        w0_sb = wpool.tile([128, NJ, 2, Z_CHUNK], f8, name="w0_sb")
        nc.sync.dma_start(w0_sb[:], wt_d[:, 0])
        x1_sb = xpool.tile([128, 2, B_CORE], f8, name="x1_sb")
        nc.sync.dma_start(x1_sb[:], xt_d[:, 1])
        tl_sb = wpool.tile([K_TAIL, DIM + B_CORE], f16, name="tl_sb")
        nc.sync.dma_start(tl_sb[:], tl_d[:])
        x2_sb = xpool.tile([128, 2, B_CORE], f8, name="x2_sb")
        nc.sync.dma_start(x2_sb[:], xt_d[:, 2])
        wr13_sb = wpool.tile([128, 3, NJ, 2, Z_CHUNK], f8, name="wr13_sb")
        nc.sync.dma_start(wr13_sb[:], wt_d[:, 1:4])
        wr47_sb = wpool.tile([128, 4, NJ, 2, Z_CHUNK], f8, name="wr47_sb")
        nc.sync.dma_start(wr47_sb[:], wt_d[:, 4:8])
        z_sb = zpool.tile([Z_CHUNK, N_ZCH * ZCOLS], bf16, name="z_sb")
        nc.sync.dma_start(z_sb[:], zt_d[:])

        po = popool.tile([128, GROUP], f32, name="po")
        # zero the never-written partitions so the single full-bank drain
        # copy below reads defined data everywhere
        nc.vector.memset(po[:], 0.0)
        out_sb = opool.tile([128, GROUP], f32, name="out_sb")

        def gsl(g):
            return slice(g * GROUP, (g + 1) * GROUP)

        def wslice(z, j):
            if z == 0:
                return w0_sb[:, j]
            if z <= 3:
                return wr13_sb[:, z - 1, j]
            return wr47_sb[:, z - 4, j]

        def xslice(j, g):
            return [x0_sb, x1_sb, x2_sb][j][:, :, gsl(g)]

        # z-outer / j / g-inner: each DoubleRow weight load serves the 4
        # batch-group matmuls, hiding LDWEIGHTS under streaming.  mm2 for
        # z is deferred until after z+2's matmul block: by then its squares
        # are long done, so the semaphore wait attached to its LDWEIGHTS
        # (move_matmul_waits_to_ldweights) never head-of-line-blocks the
        # next j-group's weight load.
        pending = []
        for z in range(N_ZCH):
            pys = [
                pypool.tile([Z_CHUNK, GROUP], f32, tag="py", name=f"py_{z}_{g}")
                for g in range(N_GROUPS)
            ]
            for j in range(NJ):
                for g in range(N_GROUPS):
                    nc.tensor.matmul(
                        pys[g][:],
                        lhsT=wslice(z, j),
                        rhs=xslice(j, g),
                        start=(j == 0),
                        stop=False,
                        perf_mode=DR,
                        skip_group_check=True,
                    )
            for g in range(N_GROUPS):
                nc.tensor.matmul(
                    pys[g][:],
                    lhsT=tl_sb[:, z * Z_CHUNK:(z + 1) * Z_CHUNK],
                    rhs=tl_sb[:, DIM + g * GROUP:DIM + (g + 1) * GROUP],
                    start=False,
                    stop=True,
                    skip_group_check=True,
                )
            if len(pending) == 2:
                pending.pop(0)()
            # Squares split across engines so PSUM banks free in parallel:
            # DVE can't read both operands from PSUM, so it copies to SBUF
            # first (the copy is what releases the py bank) and multiplies
            # there; the scalar engine squares straight from PSUM.
            sqs = [None] * N_GROUPS
            tmps = {}
            for g in (0, 2):
                tmp = tmppool.tile([Z_CHUNK, GROUP], f32, tag="tmp")
                nc.vector.tensor_copy(tmp[:], pys[g][:])
                tmps[g] = tmp
            for g in (1, 3):
                sq = sqpool.tile([Z_CHUNK, GROUP], bf16, tag="sq")
                nc.scalar.square(sq[:], pys[g][:])
                sqs[g] = sq
            for g in (0, 2):
                sq = sqpool.tile([Z_CHUNK, GROUP], bf16, tag="sq")
                nc.vector.tensor_mul(sq[:], tmps[g][:], tmps[g][:])
                sqs[g] = sq

            def make_mm2(z, sqs):
                def mm2():
                    for g in range(N_GROUPS):
                        nc.tensor.matmul(
                            po[32 * g:32 * g + N_OUT + 1, :],
                            lhsT=z_sb[:, z * ZCOLS:z * ZCOLS + N_OUT + 1],
                            rhs=sqs[g][:],
                            start=(z == 0),
                            stop=(z == N_ZCH - 1),
                            skip_group_check=True,
                            tile_position=(0, 32 * g),
                        )
                return mm2

            pending.append(make_mm2(z, sqs))
        for p in pending:
            p()

        # Final drain: one full-bank PSUM->SBUF copy, one output DMA.
        nc.vector.tensor_copy(out_sb[:], po[:])
        nc.sync.dma_start(out_d[:], out_sb[:])

    nc.finalize()
    return nc


def _get_nc():
    if "nc" not in _NC_CACHE:
        _NC_CACHE["nc"] = _build_bass()
    return _NC_CACHE["nc"]


# ----------------------------------------------------------------------------
# Host-side packing
# ----------------------------------------------------------------------------

def _pack_weights(params):
    W = _build_W(np.asarray(params, dtype=np.float32))        # [784, 1024] fp64
    W8 = (W * SW).astype(np.float32).astype(F8)               # [784, 1024]
    # wt[p, z, j, i, m] = W8[256j + 128i + p, 128z + m]
    wt = np.ascontiguousarray(
        np.asarray(W8[:NJ * 256]).reshape(NJ, 2, 128, N_ZCH, Z_CHUNK)
        .transpose(2, 3, 0, 1, 4)
    )
    # fp16 tail weights: rows 768..783 scaled by SW, plus the exact mean
    # row SX*SW*0.5*colsum(W) from centering x around 0.5
    tlw = np.zeros((K_TAIL, DIM), dtype=np.float16)
    tlw[:16] = (W[NJ * 256:] * SW).astype(np.float16)
    tlw[16] = (SX * SW * 0.5 * W.sum(axis=0)).astype(np.float16)
    return wt, tlw


def _pack_x(xc, tlw):
    """xc [2048, 784] fp32 -> (xt [128, 3, 2, 2048] fp8, tl [17, 3072] fp16)."""
    d = (xc - 0.5) * SX
    d8 = d[:, :NJ * 256].astype(F8)
    xt = np.ascontiguousarray(
        np.asarray(d8).reshape(B_CORE, NJ, 2, 128).transpose(3, 1, 2, 0)
    )
    tl = np.empty((K_TAIL, DIM + B_CORE), dtype=np.float16)
    tl[:, :DIM] = tlw
    tl[:16, DIM:] = d[:, NJ * 256:].astype(np.float16).T
    tl[16, DIM:] = 1.0
    return xt, tl


# ----------------------------------------------------------------------------
# Entry point
# ----------------------------------------------------------------------------

def kernel(input, params):
    global LAST_RESULTS
    from concourse.bass_utils import run_bass_kernel_spmd

    x = np.asarray(input, dtype=np.float32)
    wt, tlw = _pack_weights(params)
    zt = _build_Z()

    nc = _get_nc()
    in_maps = []
    for c in range(N_CORES):
        xt, tl = _pack_x(x[c * B_CORE:(c + 1) * B_CORE], tlw)
        in_maps.append({"xt": xt, "wt": wt, "tl": tl, "zt": zt})

    res = run_bass_kernel_spmd(nc, in_maps, list(range(N_CORES)), trace=TRACE)
    LAST_RESULTS = res

    outs = []
    for c in range(N_CORES):
        o = res.results[c]["out"].reshape(N_GROUPS, 32, GROUP)
        outs.append(
            (o[:, :N_OUT, :] / o[:, N_OUT:N_OUT + 1, :])
            .transpose(0, 2, 1).reshape(B_CORE, N_OUT)
        )
    return np.ascontiguousarray(np.concatenate(outs, axis=0).astype(np.float32))


# revision 22
# speedup vs baseline: 1.1717x; 1.0516x over previous
"""Trainium2 kernel for nn_Circuit_41936060678727.

The reference is a 10-qubit real-amplitude circuit (CNOT ladders + RY
rotations) applied to an amplitude-embedded batch, measured with PauliZ on
each of the 10 wires.  Every gate is linear in the state, so the whole
8-layer circuit collapses to one fixed 1024x1024 orthogonal matrix M that
depends only on `params` (8x10).  With x padded to 1024 and L2-normalized:

    out[b, p] = sum_z (x[b] @ M[:784, :])[z]^2 * sign_p(z) / ||x[b]||^2

because M is orthogonal the norm comes for free as an extra all-ones column
of the sign matrix, and the ratio is invariant to any per-tensor scaling of
the matmul inputs -- which buys the precision headroom for fp8.

Device pipeline per core (batch 16384 data-parallel over 8 cores, 2048 each):
    y = W8^T @ d8 (+ tail)  K rows 0..767: fp8e4m3 DoubleRow matmuls
                            (K=256 per pass), d8 = e4m3(32*(x-0.5)),
                            W8 = e4m3(16*W).  K rows 768..783 plus the
                            exact mean row (256*colsum(W), from centering)
                            ride in one fp16 K=17 matmul per (z, group).
    sq = y^2                pure squares, alternating vector/scalar engine
    o  = Zaug^T @ sq        bf16 matmuls accumulated into a single PSUM
                            bank at 4 col-group quadrants (one per batch
                            group of 512); the 4 quadrant matmuls pack
                            concurrently in the PE array.
Host: out = (o[:10] / o[10]) per group, concat cores.

Centering x around 0.5 halves the fp8 quantization error of the dominant
DC component; the fp16 mean row removes it from the fp8 path entirely.
"""

import numpy as np
import ml_dtypes

N_QUBITS = 10
DIM = 1 << N_QUBITS          # 1024
N_OUT = 10
D_IN = 784
B_TOTAL = 16384
N_CORES = 8
B_CORE = B_TOTAL // N_CORES  # 2048
GROUP = 512                  # batch columns per matmul (one PSUM bank, fp32)
N_GROUPS = B_CORE // GROUP   # 4
NJ = 3                       # full DoubleRow K-chunks of 256 (rows 0..767)
K_TAIL = 17                  # fp16 tail: rows 768..783 + mean row
Z_CHUNK = 128
N_ZCH = DIM // Z_CHUNK       # 8
ZCOLS = 16                   # cols 0..9 = PauliZ signs, 10 = ones, 11..15 pad
SX = 32.0                    # fp8 scale for centered x
SW = 16.0                    # fp8 scale for W

F8 = ml_dtypes.float8_e4m3   # TRN2 float8e4 (max normal 240)
BF16 = ml_dtypes.bfloat16


# ----------------------------------------------------------------------------
# Host-side precompute: collapse the circuit to W = M[:784, :]
# ----------------------------------------------------------------------------

def _apply_ry(S, theta, q):
    B = S.shape[0]
    left, right = 1 << q, 1 << (N_QUBITS - q - 1)
    s = S.reshape(B, left, 2, right)
    c, sn = np.cos(theta / 2), np.sin(theta / 2)
    s0 = c * s[:, :, 0] - sn * s[:, :, 1]
    s1 = sn * s[:, :, 0] + c * s[:, :, 1]
    return np.stack([s0, s1], axis=2).reshape(B, DIM)


def _apply_cnot(S, q):
    B = S.shape[0]
    left, right = 1 << q, 1 << (N_QUBITS - q - 2)
    s = S.reshape(B, left, 2, 2, right)
    s = np.concatenate([s[:, :, :1], np.flip(s[:, :, 1:], axis=3)], axis=2)
    return s.reshape(B, DIM)


def _build_W(params):
    """Circuit applied to basis rows e_0..e_783 -> W[784, 1024], fp64."""
    w = np.pi * np.tanh(params.astype(np.float64))
    S = np.zeros((D_IN, DIM), dtype=np.float64)
    S[np.arange(D_IN), np.arange(D_IN)] = 1.0
    for l in range(params.shape[0]):
        for start in (0, 1):
            for i in range(start, N_QUBITS - 1, 2):
                S = _apply_cnot(S, i)
        for i in range(N_QUBITS):
            S = _apply_ry(S, w[l, i], i)
    return S


def _build_Z():
    z = np.arange(DIM)
    Z = np.zeros((DIM, ZCOLS), dtype=np.float32)
    for p in range(N_OUT):
        Z[:, p] = 1.0 - 2.0 * ((z >> (N_QUBITS - 1 - p)) & 1)
    Z[:, N_OUT] = 1.0
    # device layout [128, 8*16]: z-chunk zc rows zc*128.. at cols zc*16..
    Zd = Z.reshape(N_ZCH, Z_CHUNK, ZCOLS).transpose(1, 0, 2).reshape(Z_CHUNK, -1)
    return np.ascontiguousarray(Zd).astype(BF16)


# ----------------------------------------------------------------------------
# Bass program (identical SPMD program on all 8 cores)
# ----------------------------------------------------------------------------

_NC_CACHE = {}
TRACE = False           # test harness can flip this for profiling
LAST_RESULTS = None


def _build_bass():
    from contextlib import ExitStack

    import concourse.tile as tile
    from concourse import bacc, mybir

    f32 = mybir.dt.float32
    f8 = mybir.dt.float8e4
    f16 = mybir.dt.float16
    bf16 = mybir.dt.bfloat16
    DR = mybir.MatmulPerfMode.DoubleRow

    nc = bacc.Bacc(
        "TRN2", target_bir_lowering=False, debug=False, num_devices=N_CORES
    )
    xt_d = nc.declare_dram_parameter("xt", [128, NJ, 2, B_CORE], f8, isOutput=False)
    wt_d = nc.declare_dram_parameter(
        "wt", [128, N_ZCH, NJ, 2, Z_CHUNK], f8, isOutput=False
    )
    # fp16 tail operands packed in one tensor: cols 0..1023 = weights
    # (16 W rows + mean row), cols 1024.. = rhs (16 x rows + ones row)
    tl_d = nc.declare_dram_parameter(
        "tl", [K_TAIL, DIM + B_CORE], f16, isOutput=False
    )
    zt_d = nc.declare_dram_parameter(
        "zt", [Z_CHUNK, N_ZCH * ZCOLS], bf16, isOutput=False
    )
    out_d = nc.declare_dram_parameter("out", [128, GROUP], f32, isOutput=True)

    with ExitStack() as ctx:
        tc = ctx.enter_context(tile.TileContext(nc))
        wpool = ctx.enter_context(tc.tile_pool(name="w", bufs=1))
        xpool = ctx.enter_context(tc.tile_pool(name="x", bufs=1))
        zpool = ctx.enter_context(tc.tile_pool(name="z", bufs=1))
        sqpool = ctx.enter_context(tc.tile_pool(name="sq", bufs=12))
        tmppool = ctx.enter_context(tc.tile_pool(name="tmp", bufs=3))
        opool = ctx.enter_context(tc.tile_pool(name="osb", bufs=1))
        # 7 rotating y banks + 1 persistent output bank = all 8 PSUM banks
        pypool = ctx.enter_context(tc.tile_pool(name="py", bufs=7, space="PSUM"))
        popool = ctx.enter_context(tc.tile_pool(name="po", bufs=1, space="PSUM"))

        # PE pre-warm: dependency-free full-K bf16 matmuls keep the tensor
        # engine ARRAY visibly busy during the DMA prefix so the HAM
        # clock-gate releases (1.2 -> 2.4 GHz) before real work arrives.
        # (K=1 matmuls barely register as activity to the HAM.)
        warm_in = opool.tile([128, 384], bf16, name="warm_in")
        nc.any.memset(warm_in[:], 0.0)
        warm_ps = pypool.tile([128, 256], f32, name="warm_ps", tag="py")
        for i in range(8):
            nc.tensor.matmul(
                warm_ps[:],
                lhsT=warm_in[:, 0:128],
                rhs=warm_in[:, 128:384],
                start=True,
                stop=True,
                skip_group_check=True,
            )

        # Input DMAs across BOTH HWDGE rings (sync + scalar) so descriptor
        # generation (~700ns per dma_start, serialized per queue) runs in
        # parallel.  The x path (critical for the first matmuls) goes on
        # sync, most-critical first and j0 split in half; the weight bulk
        # rides the scalar ring, whose queue is otherwise idle early.
        w0_sb = wpool.tile([128, NJ, 2, Z_CHUNK], f8, name="w0_sb")
        nc.sync.dma_start(w0_sb[:], wt_d[:, 0])
        x0a_sb = xpool.tile([128, 2, B_CORE // 2], f8, name="x0a_sb")
        nc.sync.dma_start(x0a_sb[:], xt_d[:, 0, :, :B_CORE // 2])
        x0b_sb = xpool.tile([128, 2, B_CORE // 2], f8, name="x0b_sb")
        nc.sync.dma_start(x0b_sb[:], xt_d[:, 0, :, B_CORE // 2:])
        x1_sb = xpool.tile([128, 2, B_CORE], f8, name="x1_sb")
        nc.sync.dma_start(x1_sb[:], xt_d[:, 1])
        tl_sb = wpool.tile([K_TAIL, DIM + B_CORE], f16, name="tl_sb")
        nc.sync.dma_start(tl_sb[:], tl_d[:])
        x2_sb = xpool.tile([128, 2, B_CORE], f8, name="x2_sb")
        nc.sync.dma_start(x2_sb[:], xt_d[:, 2])
        wr13_sb = wpool.tile([128, 3, NJ, 2, Z_CHUNK], f8, name="wr13_sb")
        nc.scalar.dma_start(wr13_sb[:], wt_d[:, 1:4])
        wr47_sb = wpool.tile([128, 4, NJ, 2, Z_CHUNK], f8, name="wr47_sb")
        nc.scalar.dma_start(wr47_sb[:], wt_d[:, 4:8])
        z_sb = zpool.tile([Z_CHUNK, N_ZCH * ZCOLS], bf16, name="z_sb")
        nc.scalar.dma_start(z_sb[:], zt_d[:])

        po = popool.tile([128, GROUP], f32, name="po")
        # zero the never-written partitions so the single full-bank drain
        # copy below reads defined data everywhere
        nc.vector.memset(po[:], 0.0)
        out_sb = opool.tile([128, GROUP], f32, name="out_sb")

        def gsl(g):
            return slice(g * GROUP, (g + 1) * GROUP)

        def wslice(z, j):
            if z == 0:
                return w0_sb[:, j]
            if z <= 3:
                return wr13_sb[:, z - 1, j]
            return wr47_sb[:, z - 4, j]

        def xslice(j, g):
            if j == 0:
                t = x0a_sb if g < 2 else x0b_sb
                return t[:, :, (g % 2) * GROUP:(g % 2 + 1) * GROUP]
            return (x1_sb if j == 1 else x2_sb)[:, :, gsl(g)]

        # z-outer / j / g-inner: each DoubleRow weight load serves the 4
        # batch-group matmuls, hiding LDWEIGHTS under streaming.  mm2 for
        # z is deferred until after z+2's matmul block: by then its squares
        # are long done, so the semaphore wait attached to its LDWEIGHTS
        # (move_matmul_waits_to_ldweights) never head-of-line-blocks the
        # next j-group's weight load.
        pending = []
        for z in range(N_ZCH):
            pys = [
                pypool.tile([Z_CHUNK, GROUP], f32, tag="py", name=f"py_{z}_{g}")
                for g in range(N_GROUPS)
            ]
            for j in range(NJ):
                for g in range(N_GROUPS):
                    nc.tensor.matmul(
                        pys[g][:],
                        lhsT=wslice(z, j),
                        rhs=xslice(j, g),
                        start=(j == 0),
                        stop=False,
                        perf_mode=DR,
                        skip_group_check=True,
                    )
            for g in range(N_GROUPS):
                nc.tensor.matmul(
                    pys[g][:],
                    lhsT=tl_sb[:, z * Z_CHUNK:(z + 1) * Z_CHUNK],
                    rhs=tl_sb[:, DIM + g * GROUP:DIM + (g + 1) * GROUP],
                    start=False,
                    stop=True,
                    skip_group_check=True,
                )
            if len(pending) == 2:
                pending.pop(0)()
            # Squares split across engines so PSUM banks free in parallel:
            # DVE can't read both operands from PSUM, so it copies to SBUF
            # first (the copy is what releases the py bank) and multiplies
            # there; the scalar engine squares straight from PSUM.
            sqs = [None] * N_GROUPS
            tmps = {}
            for g in (0, 2):
                tmp = tmppool.tile([Z_CHUNK, GROUP], f32, tag="tmp")
                nc.vector.tensor_copy(tmp[:], pys[g][:])
                tmps[g] = tmp
            for g in (1, 3):
                sq = sqpool.tile([Z_CHUNK, GROUP], bf16, tag="sq")
                nc.scalar.square(sq[:], pys[g][:])
                sqs[g] = sq
            for g in (0, 2):
                sq = sqpool.tile([Z_CHUNK, GROUP], bf16, tag="sq")
                nc.vector.tensor_mul(sq[:], tmps[g][:], tmps[g][:])
                sqs[g] = sq

            def make_mm2(z, sqs):
                def mm2():
                    for g in range(N_GROUPS):
                        nc.tensor.matmul(
                            po[32 * g:32 * g + N_OUT + 1, :],
                            lhsT=z_sb[:, z * ZCOLS:z * ZCOLS + N_OUT + 1],
                            rhs=sqs[g][:],
                            start=(z == 0),
                            stop=(z == N_ZCH - 1),
                            skip_group_check=True,
                            tile_position=(0, 32 * g),
                        )
                return mm2

            pending.append(make_mm2(z, sqs))
        for p in pending:
            p()

        # Final drain: one full-bank PSUM->SBUF copy, one output DMA.
        nc.vector.tensor_copy(out_sb[:], po[:])
        nc.sync.dma_start(out_d[:], out_sb[:])

    nc.finalize()
    return nc


def _get_nc():
    if "nc" not in _NC_CACHE:
        _NC_CACHE["nc"] = _build_bass()
    return _NC_CACHE["nc"]


# ----------------------------------------------------------------------------
# Host-side packing
# ----------------------------------------------------------------------------

def _pack_weights(params):
    W = _build_W(np.asarray(params, dtype=np.float32))        # [784, 1024] fp64
    W8 = (W * SW).astype(np.float32).astype(F8)               # [784, 1024]
    # wt[p, z, j, i, m] = W8[256j + 128i + p, 128z + m]
    wt = np.ascontiguousarray(
        np.asarray(W8[:NJ * 256]).reshape(NJ, 2, 128, N_ZCH, Z_CHUNK)
        .transpose(2, 3, 0, 1, 4)
    )
    # fp16 tail weights: rows 768..783 scaled by SW, plus the exact mean
    # row SX*SW*0.5*colsum(W) from centering x around 0.5
    tlw = np.zeros((K_TAIL, DIM), dtype=np.float16)
    tlw[:16] = (W[NJ * 256:] * SW).astype(np.float16)
    tlw[16] = (SX * SW * 0.5 * W.sum(axis=0)).astype(np.float16)
    return wt, tlw


def _pack_x(xc, tlw):
    """xc [2048, 784] fp32 -> (xt [128, 3, 2, 2048] fp8, tl [17, 3072] fp16)."""
    d = (xc - 0.5) * SX
    d8 = d[:, :NJ * 256].astype(F8)
    xt = np.ascontiguousarray(
        np.asarray(d8).reshape(B_CORE, NJ, 2, 128).transpose(3, 1, 2, 0)
    )
    tl = np.empty((K_TAIL, DIM + B_CORE), dtype=np.float16)
    tl[:, :DIM] = tlw
    tl[:16, DIM:] = d[:, NJ * 256:].astype(np.float16).T
    tl[16, DIM:] = 1.0
    return xt, tl


# ----------------------------------------------------------------------------
# Entry point
# ----------------------------------------------------------------------------

def kernel(input, params):
    global LAST_RESULTS
    from concourse.bass_utils import run_bass_kernel_spmd

    x = np.asarray(input, dtype=np.float32)
    wt, tlw = _pack_weights(params)
    zt = _build_Z()

    nc = _get_nc()
    in_maps = []
    for c in range(N_CORES):
        xt, tl = _pack_x(x[c * B_CORE:(c + 1) * B_CORE], tlw)
        in_maps.append({"xt": xt, "wt": wt, "tl": tl, "zt": zt})

    res = run_bass_kernel_spmd(nc, in_maps, list(range(N_CORES)), trace=TRACE)
    LAST_RESULTS = res

    outs = []
    for c in range(N_CORES):
        o = res.results[c]["out"].reshape(N_GROUPS, 32, GROUP)
        outs.append(
            (o[:, :N_OUT, :] / o[:, N_OUT:N_OUT + 1, :])
            .transpose(0, 2, 1).reshape(B_CORE, N_OUT)
        )
    return np.ascontiguousarray(np.concatenate(outs, axis=0).astype(np.float32))
